# revision 40
# baseline (speedup 1.0000x reference)
"""NeuralMMU Trainium2 kernel — ACT+DVE split gelu, fused scan-pack.

Per core: 131072 addrs, 64 iterations x 2048 addrs.  The baseline ran
every Gelu on the scalar engine (ACT) and a 3-op threshold/pack chain
on DVE, leaving ACT the bottleneck at ~1.89us/iter.  This version
offloads 384 gelu columns per iteration to DVE via custom DVE ops and
collapses the whole threshold+weight+pack pipeline into one custom DVE
instruction, running ~1.91us/iter with ACT and DVE co-saturated:

  PE    L1: 4 matmuls k=66 (W1 2-way bf16 split rows 0..63 + b1 2-way
        bf16 split rows 64..65 against all-ones plane rows) with fp8
        e4m3 bit planes moving (bits/ones are exact in fp8) -> 2048
        pre-activation cols incl. bias.  L2: 16 matmuls, stationary
        h-chunk f32, moving W2' f32 [128,26] where W2' = W2/(0.5-b2)
        per logit column, so the downstream threshold becomes the
        hardware constant One.  Logits land in the TAIL 416 cols
        (bank 3) of the half being freed; the L1 tail segment of t+2
        carries the WAR on pack(t) with ~2 iterations of slack.
  ACT   ACT-B: one Gelu on cols [512,2048) (banks 1-3 — bank-disjoint
        from DVE's reads: PSUM reads from different engines serialize
        at bank granularity).  ACT-A: [384,512) after the DVE passes
        release bank 0.  No bias (b1 lives in PSUM via the ones rows).
  DVE   cols [0,384) of bank 0: 2-pass polynomial gelu (GELU_P1/P2:
        p=(((c0 v+c1)v+c2)v+c3)v with v=x^2, h=q0 p^2+p+(c6 x+d)x;
        fit rms 1.6e-6 vs exact gelu over the actual pre range).
        pack: ONE custom op per iter: out=scan(add, select(l'>1, 2^j, 0))
        over the 416 logit cols, with a stride-0 inner write AP so only
        the 32 page-end cumulative sums land (contiguous); the host
        takes adjacent differences to recover the 13-bit packed halves.
  DMA   in: [66, 4096] fp8 planes per 2 iters via the gpsimd SWDGE
        queue (8.4 MB/core); out: [128, 512] f32 per 16 iters via SP.

Numerics: exact except W1/b1 2-way bf16 split (~2^-18), the ACT Gelu
table, and the DVE gelu fit (1.6e-6 rms) on 19% of addrs; 9/1M
mismatched addrs, rel err 2.0e-4.

TimelineSim: 132725 ns (prior baseline 134711 ns).
"""

import numpy as np
from contextlib import ExitStack

import concourse.bass as bass
import concourse.mybir as mybir
import concourse.tile as tile
from concourse import bacc, bass_utils
from concourse import dve_ops as DO
from concourse.dve_spec import (
    Spec, Src0, Src1, C0, C1, C2, C3, Zero, One,
    sq, select, lower, AluOp, scan,
    _has_src1 as _has_src1, _spill_c3_to_src1,
)
from concourse.dve_uop import DveOpSpec
from concourse.ap import AP

B = 1_048_576
NCORES = 8
PER = B // NCORES          # 131072 addrs per core
BLK = 512                  # addrs per L1 PE block
NBLK = 4                   # L1 blocks per iteration
CH = 128                   # addrs per L2 chunk (stationary width)
NCH = 16                   # L2 chunks per iteration
CHUNK = NBLK * BLK         # 2048 addrs per iteration
N_ITERS = PER // CHUNK     # 64
GIN = 2                    # iters per input DMA
GOUT = 16                  # iters per output DMA
NLOG = 26                  # logits per addr
LW = NCH * NLOG            # 416 logit cols per iteration
KP = 66                    # L1 contraction: 2x32 bit rows + 2 bias rows
DCOLS = 384                # gelu cols computed on DVE per iteration (bank 0)
ABCOL = 512                # ACT-B covers [ABCOL, CHUNK) (banks 1-3)
# ACT-A covers [DCOLS, ABCOL) after the DVE passes release bank 0

F32 = mybir.dt.float32
BF16 = mybir.dt.bfloat16
FP8 = mybir.dt.float8e4
AF = mybir.ActivationFunctionType

# COMP12 gelu fit (q1 normalized to 1, constant dropped):
#   p = (((GC0*v + GC1)*v + GC2)*v + GC3)*v,  v = x*x
#   h = GQ0*p*p + p + (GC6*x + GD)*x
GC0 = -9.22404584e-05
GC1 = 3.22352572e-03
GC2 = -4.67451012e-02
GC3 = 2.77984829e-01
GQ0 = -2.53889672e-01
GC6 = 1.20921287e-01
GD = 5.00000111e-01


# --------------------------------------------------------------------------
# custom DVE ops (registered once per process)
# --------------------------------------------------------------------------

def _register(name, spec, subdim=False):
    for o in DO.OPS:
        if o.name == name:
            return o
    row = DO._CUSTOM_DVE_ROW_BASE + len(DO.OPS)
    DO._SUB_OPCODE_FOR_NAME[name] = row
    c = DveOpSpec(name=name, opcode=row, uops=lower(spec, ver="v3"),
                  rd1_en=_has_src1(spec))
    op = DO.DveOp(name, spec, subdim=subdim, uops_sha={"v3": c.sha("v3")})
    DO.OPS.append(op)
    DO.CUSTOM_DVE_SPECS[name] = spec
    return op


def _ref_pack(in0, in1, s0, s1, imm2):
    val = np.where(in0.reshape(in0.shape[0], -1) > 1.0,
                   in1.astype(np.float64).reshape(in0.shape[0], -1), 0.0)
    return np.add.accumulate(val, axis=-1).astype(np.float32).reshape(in0.shape)


def _ref_g1(in0, in1, s0, s1, imm2):
    c3 = in1[:, :1].astype(np.float64)
    v = in0.astype(np.float64) ** 2
    return ((((s0 * v + s1) * v + imm2) * v + c3) * v).astype(np.float32)


def _ref_g2(in0, in1, s0, s1, imm2):
    p = in1.astype(np.float64)
    x = in0.astype(np.float64)
    return (s0 * p * p + p + (s1 * x + imm2) * x).astype(np.float32)


PACK_OP = _register("PACK_SCAN_ANT", Spec(
    body=scan(AluOp.ADD, select(Src0 > One, Src1, Zero)),
    reference=_ref_pack))

_v1 = sq(Src0)
GELU_P1 = _register("GELU_P1_ANT", Spec(
    body=_spill_c3_to_src1((((C0 * _v1 + C1) * _v1 + C2) * _v1 + C3) * _v1),
    reference=_ref_g1))

_sp = sq(Src1)
GELU_P2 = _register("GELU_P2_ANT", Spec(
    body=(_sp * C0 + Src1) + (Src0 * C1 + C2) * Src0,
    reference=_ref_g2))


# --------------------------------------------------------------------------
# device module
# --------------------------------------------------------------------------

def build_nc(n_iters: int = N_ITERS, act=AF.Gelu) -> bass.Bass:
    nc = bacc.Bacc("TRN2")
    assert n_iters % GOUT == 0 and n_iters % GIN == 0

    bp = nc.dram_tensor("bp", [n_iters // GIN, KP, GIN * CHUNK], FP8,
                        kind="ExternalInput")
    w1s_d = nc.dram_tensor("w1s", [KP, CH], BF16, kind="ExternalInput")
    w2s_d = nc.dram_tensor("w2s", [CH, NLOG], F32, kind="ExternalInput")
    wv_d = nc.dram_tensor("wv", [CH, LW], F32, kind="ExternalInput")
    c3_d = nc.dram_tensor("c3c", [CH, 1], F32, kind="ExternalInput")
    outp = nc.dram_tensor("outp", [n_iters // GOUT, CH, GOUT * 32], F32,
                          kind="ExternalOutput")

    with ExitStack() as ctx:
        tc = ctx.enter_context(tile.TileContext(nc))
        const = ctx.enter_context(tc.tile_pool(name="const", bufs=1))
        rpool = ctx.enter_context(tc.tile_pool(name="rp", bufs=3))
        ppool = ctx.enter_context(
            tc.tile_pool(name="ppool", bufs=1, space="PSUM"))
        hp = ctx.enter_context(tc.tile_pool(name="hp", bufs=2))
        ptp = ctx.enter_context(tc.tile_pool(name="ptp", bufs=2))
        pksp = ctx.enter_context(tc.tile_pool(name="pksp", bufs=2))

        # one persistent 8-bank psum tensor; all deps are subtile-based
        PP = ppool.tile([128, 2 * CHUNK], F32, name="PP")

        w1s = const.tile([KP, CH], BF16, name="w1s")
        w2s = const.tile([CH, NLOG], F32, name="w2s")
        wv = const.tile([CH, LW], F32, name="wv")
        c3c = const.tile([CH, 1], F32, name="c3c")

        R = {}
        hs = {}
        pks = None

        def half(t):
            return CHUNK * (t % 2)

        def load_input(g):
            if g < n_iters // GIN and g not in R:
                r = rpool.tile([KP, GIN * CHUNK], FP8, name="r")
                nc.gpsimd.dma_start(r[:], bp[g])
                R[g] = r

        def l1seg(t, s0, s1):
            r = R[t // GIN]
            c0 = CHUNK * (t % GIN) + s0
            nc.tensor.matmul(
                PP[:, half(t) + s0:half(t) + s1],
                w1s[:],
                r[:, c0:c0 + (s1 - s0)],
                start=True, stop=True, tile_position=(0, 0),
            )

        def l1(t, segs):
            if t >= n_iters:
                return
            for s0, s1 in segs:
                l1seg(t, s0, s1)

        def gelu(t):
            """DVE: cols [0, DCOLS); ACT-B: [ABCOL, CHUNK); ACT-A:
            [DCOLS, ABCOL) after the DVE passes release bank 0."""
            h = hp.tile([128, CHUNK], F32, name="h")
            nc.scalar.activation(h[:, ABCOL:CHUNK],
                                 PP[:, half(t) + ABCOL:half(t) + CHUNK],
                                 act, scale=1.0)
            pt = ptp.tile([128, DCOLS], F32, name="pt")
            nc.vector._custom_dve(
                GELU_P1, out=pt[:],
                in0=PP[:, half(t):half(t) + DCOLS],
                in1=c3c[:], s0=GC0, s1=GC1, imm2=GC2)
            nc.vector._custom_dve(
                GELU_P2, out=h[:, 0:DCOLS],
                in0=PP[:, half(t):half(t) + DCOLS],
                in1=pt[:], s0=GQ0, s1=GC6, imm2=GD)
            nc.scalar.activation(h[:, DCOLS:ABCOL],
                                 PP[:, half(t) + DCOLS:half(t) + ABCOL],
                                 act, scale=1.0)
            hs[t] = h

        LOFF = CHUNK - LW      # logits live in the tail of the half (bank 3)

        def l2(t):
            h = hs.pop(t)
            # ACT-B chunks first (ready earliest), then DVE, then ACT-A
            for c in list(range(4, NCH)) + [0, 1, 2, 3]:
                o = LOFF + NLOG * c
                nc.tensor.matmul(
                    PP[:, half(t) + o:half(t) + o + NLOG],
                    h[:, CH * c:CH * (c + 1)],
                    w2s[:],
                    start=True, stop=True, tile_position=(0, 0),
                )

        def pack1(t, lo, hi):
            """Pack pages lo..hi (of 32) of iter t's logits."""
            base = pks[:, 32 * (t % GOUT) + lo:32 * (t % GOUT) + hi]
            out_ap = AP(base.tensor, base.offset,
                        [list(base.ap[0]), [1, hi - lo], [0, 13]])
            nc.vector._custom_dve(
                PACK_OP, out=out_ap,
                in0=PP[:, half(t) + LOFF + 13 * lo:half(t) + LOFF + 13 * hi]
                .rearrange("p (s n) -> p s n", n=13),
                in1=wv[:, 0:13 * (hi - lo)])

        def pack(t):
            nonlocal pks
            if t % GOUT == 0:
                pks = pksp.tile([128, GOUT * 32], F32, name="pks")
            if t == n_iters - 1:
                pack1(t, 0, 32)
                nc.sync.dma_start(outp[t // GOUT][:, 32 * (GOUT - 1):],
                                  pks[:, 32 * (GOUT - 1):])
                return
            pack1(t, 0, 32)
            if t == n_iters - 2:
                nc.sync.dma_start(outp[t // GOUT][:, 0:32 * (GOUT - 1)],
                                  pks[:, 0:32 * (GOUT - 1)])
            elif t % GOUT == GOUT - 1:
                nc.sync.dma_start(outp[t // GOUT], pks[:])

        # Warm the ACT gelu table and the PE p-state during the first
        # input DMAs.
        warm = const.tile([128, BLK], BF16, name="warm")
        nc.gpsimd.memset(warm[:], 0.0)
        warmo = const.tile([128, 1], F32, name="warmo")
        nc.scalar.activation(warmo[:], warm[:, 0:1], act, scale=1.0)
        for _ in range(5):
            nc.tensor.matmul(
                PP[0:1, 0:BLK], warm[0:1, 0:1], warm[0:1, 0:BLK],
                start=True, stop=True, tile_position=(0, 0),
            )

        # Prologue: group 0 split so l1(0) can start early.
        r0 = rpool.tile([KP, GIN * CHUNK], FP8, name="r")
        nc.sync.dma_start(r0[:, 0:CHUNK], bp[0, :, 0:CHUNK])
        nc.sync.dma_start(w1s[:], w1s_d[:])
        nc.sync.dma_start(w2s[:], w2s_d[:])
        nc.sync.dma_start(wv[:], wv_d[:])
        nc.sync.dma_start(c3c[:], c3_d[:])
        nc.gpsimd.dma_start(r0[:, CHUNK:GIN * CHUNK], bp[0, :, CHUNK:GIN * CHUNK])
        R[0] = r0
        load_input(1)

        SEG_HEAD = [(0, BLK), (BLK, 2 * BLK), (2 * BLK, 3 * BLK)]
        SEG_TAIL = [(3 * BLK, 4 * BLK)]   # overwrites logits: waits pack(t-1)

        l1(0, SEG_HEAD + SEG_TAIL)

        for t in range(n_iters):
            gelu(t)
            if t >= 1:
                l2(t - 1)
                pack(t - 1)
            if t % GIN == 0:
                load_input(t // GIN + 2)
            l1(t + 1, SEG_HEAD)
            l1(t + 1, SEG_TAIL)

        l2(n_iters - 1)
        pack(n_iters - 1)

    return nc


# --------------------------------------------------------------------------
# host-side packing / unpacking
# --------------------------------------------------------------------------

def make_const_inputs(W1, b1, W2, b2):
    import ml_dtypes

    w1 = np.asarray(W1[0:32, :], dtype=np.float32)
    hi = w1.astype(ml_dtypes.bfloat16)
    lo = (w1 - hi.astype(np.float32)).astype(ml_dtypes.bfloat16)
    b1f = np.asarray(b1, dtype=np.float32)
    b1hi = b1f.astype(ml_dtypes.bfloat16)
    b1lo = (b1f - b1hi.astype(np.float32)).astype(ml_dtypes.bfloat16)
    w1s = np.zeros((KP, CH), dtype=ml_dtypes.bfloat16)
    w1s[0:32] = hi
    w1s[32:64] = lo
    w1s[64] = b1hi
    w1s[65] = b1lo

    th = 0.5 - np.asarray(b2[:NLOG], dtype=np.float64)
    w2s = (np.asarray(W2[:, :NLOG], dtype=np.float64) / th[None, :])
    w2s = w2s.astype(np.float32)
    flipmask = np.int64(0)
    for j in range(NLOG):
        if th[j] < 0:
            flipmask |= np.int64(1) << j

    wv = np.tile(2.0 ** np.arange(13, dtype=np.float32), LW // 13)
    wvt = np.broadcast_to(wv[None, :], (CH, LW)).copy()

    c3c = np.full((CH, 1), GC3, dtype=np.float32)
    return {"w1s": w1s, "w2s": w2s, "wv": wvt, "c3c": c3c}, flipmask


def make_bit_planes(virtual_addr, n_iters: int = N_ITERS):
    """Per-core [n_iters//GIN, 66, GIN*2048] fp8 planes: rows 0..31 and
    32..63 both hold bit k of each addr column; rows 64..65 are ones
    (b1 2-split rides the stationary)."""
    import ml_dtypes

    va32 = np.asarray(virtual_addr).astype(np.uint32)
    per = n_iters * CHUNK
    ncores = va32.size // per
    ONE8 = np.uint8(0x38)          # 1.0 in float8_e4m3fn
    out = []
    for c in range(ncores):
        seg = va32[c * per:(c + 1) * per]
        byt = seg.view(np.uint8).reshape(n_iters // GIN, GIN * CHUNK, 4)
        bits = np.unpackbits(byt, axis=-1, bitorder="little")
        pl = bits.transpose(0, 2, 1)           # [tt, 32, n]
        u8 = np.empty((n_iters // GIN, KP, GIN * CHUNK), dtype=np.uint8)
        u8[:, 0:32] = pl * ONE8
        u8[:, 32:64] = u8[:, 0:32]
        u8[:, 64:66] = ONE8
        out.append(u8.view(ml_dtypes.float8_e4m3fn))
    return out


def combine_output(o, flipmask, n_iters: int = N_ITERS):
    """[n_iters//GOUT, 128, GOUT*32] f32 cumulative page sums -> [per] int64.

    col 32*ts + s holds the cumulative packed sum through page s of iter
    GOUT*tt + ts; page s = (chunk s//2, half s%2); adjacent differences
    give the 13-bit halves; addr = CHUNK*t + CH*c + p."""
    arr = np.asarray(o, dtype=np.float64).reshape(
        n_iters // GOUT, CH, GOUT, 32)
    d = np.empty_like(arr)
    d[..., 0] = arr[..., 0]
    d[..., 1:] = arr[..., 1:] - arr[..., :-1]
    v = np.rint(d).astype(np.int64).reshape(n_iters // GOUT, CH, GOUT, NCH, 2)
    val = (v[..., 0] + 8192 * v[..., 1]) ^ flipmask   # [tt, p, ts, c]
    return val.transpose(0, 2, 3, 1).reshape(-1).copy()


_NC_CACHE = {}
TRACE = False
LAST_RES = None


def kernel(virtual_addr, W1, b1, W2, b2):
    global LAST_RES
    if "nc" not in _NC_CACHE:
        nc = build_nc(N_ITERS)
        nc.finalize()
        _NC_CACHE["nc"] = nc
    nc = _NC_CACHE["nc"]

    consts, flipmask = make_const_inputs(W1, b1, W2, b2)
    planes = make_bit_planes(virtual_addr, N_ITERS)
    in_maps = [{"bp": planes[c], **consts} for c in range(NCORES)]

    res = bass_utils.run_bass_kernel_spmd(
        nc, in_maps, list(range(NCORES)), trace=TRACE
    )
    LAST_RES = res

    outs = [combine_output(res.results[c]["outp"], flipmask)
            for c in range(NCORES)]
    return np.concatenate(outs)
